# revision 3
# baseline (speedup 1.0000x reference)
"""PitchAutoCorrelator Trainium2 kernel.

x [64, 1, 320000] f32, periods [64, 4000] int -> acorr [64, 1, 4000, 5] f32.

Per 80-sample frame f with period p: the 5 outputs are normalized
correlations between the frame and the lagged window
x[f*80 - p - 2 + w : ... + 80] for w in 0..4 (zero-padded at row start,
edge-clamped at row end, matching the reference's clipped gather).

Sharding: pure batch data-parallel, 8 rows per NeuronCore.
Per core: frames are blocked [128 partitions x 250 windows]; lag windows
(84 contiguous f32 at a data-dependent offset) are fetched with
one-descriptor-per-partition indirect DMAs (the only indexing form the
HW honors); dot products / energies on DVE+ACT; epilogue uses
acorr = dotp * exp(-0.5*ln(frame_nrg*lag_nrg + 1e-9)).
"""

import sys

if "/opt/trn_rl_repo" not in sys.path:
    sys.path.insert(0, "/opt/trn_rl_repo")

import numpy as np

FRAME = 80
PMAX = 300
WIN = 84          # lag window length (80 + 2*RADIUS)
W = 5             # number of lags
B, N, F = 64, 320000, 4000
NCORES = 8
BS = B // NCORES              # batch rows per core
ROWPAD = 302 + N + 2          # per-row padded length
L = BS * ROWPAD               # padded signal length per core
FPC = BS * F                  # frames per core (32000)
P = 128                       # partitions
JTOT = FPC // P               # windows per partition (250)
NSB = 2                       # super-blocks of [128 x 125]
JSB = JTOT // NSB             # 125
K = 25                        # windows per chunk
NCHUNK = JTOT // K            # 10 chunks

_cache = {}


def _build():
    import concourse.bass as bass
    import concourse.bacc as bacc
    import concourse.mybir as mybir
    import concourse.tile as tile

    dt = mybir.dt
    nc = bacc.Bacc("TRN2", target_bir_lowering=False, debug=False)
    xp = nc.dram_tensor("xp", [L], dt.float32, kind="ExternalInput")
    per = nc.dram_tensor("per", [FPC], dt.int32, kind="ExternalInput").ap()
    cw = nc.dram_tensor("cw", [P, JTOT], dt.int32, kind="ExternalInput").ap()
    out = nc.dram_tensor("out", [FPC, W], dt.float32, kind="ExternalOutput").ap()

    xp_slide = bass.AP(tensor=xp, offset=0, ap=[[1, L - WIN + 1], [1, WIN]])

    with tile.TileContext(nc) as tc:
        with tc.tile_pool(name="sbuf", bufs=2) as pool, \
             tc.tile_pool(name="cpool", bufs=1) as cpool:
            cw_t = cpool.tile([P, JTOT], dt.int32)
            nc.sync.dma_start(out=cw_t[:], in_=cw[:, :])
            eps_t = cpool.tile([P, 1], dt.float32)
            nc.vector.memset(eps_t[:], 1e-9)

            for ch in range(NCHUNK):
                sb, kk = ch // (JSB // K), ch % (JSB // K)
                jbase = sb * JSB + kk * K          # column into cw
                gbase = sb * (JSB * P) + kk * K    # frame-index base

                # periods for this chunk: per[gbase + 125*p + k]
                per_t = pool.tile([P, K], dt.int32, tag="per")
                nc.sync.dma_start(
                    out=per_t[:],
                    in_=bass.AP(tensor=per.tensor, offset=gbase,
                                ap=[[JSB, P], [1, K]]),
                )
                idx_t = pool.tile([P, K], dt.int32, tag="idx")
                nc.vector.tensor_tensor(
                    out=idx_t[:], in0=cw_t[:, jbase:jbase + K], in1=per_t[:],
                    op=mybir.AluOpType.subtract,
                )

                # lag windows: one indirect DMA per window column
                lag_t = pool.tile([P, K * WIN], dt.float32, tag="lag")
                for k in range(K):
                    nc.gpsimd.indirect_dma_start(
                        out=lag_t[:, k * WIN:(k + 1) * WIN],
                        out_offset=None,
                        in_=xp_slide,
                        in_offset=bass.IndirectOffsetOnAxis(
                            ap=idx_t[:, k:k + 1], axis=1),
                    )

                # frames: 4 row-group DMAs (row stride changes every 32
                # partitions)
                fr_t = pool.tile([P, K * FRAME], dt.float32, tag="fr")
                for rg in range(4):
                    g0 = sb * (JSB * P) + (32 * rg) * JSB + kk * K
                    r = sb * 4 + rg
                    off = g0 * FRAME + 304 * r + 302
                    nc.sync.dma_start(
                        out=fr_t[32 * rg:32 * rg + 32, :],
                        in_=bass.AP(tensor=xp, offset=off,
                                    ap=[[JSB * FRAME, 32], [1, K * FRAME]]),
                    )

                lag_v = lag_t[:].rearrange("p (k n) -> p k n", k=K)
                fr_v = fr_t[:].rearrange("p (k n) -> p k n", k=K)

                # dot products: 5 shifted elementwise mults + add tree
                pv_t = pool.tile([P, W * K * FRAME], dt.float32, tag="pv")
                pv = pv_t[:].rearrange("p (w k n) -> p w k n", w=W, k=K)
                for w in range(W):
                    nc.vector.tensor_tensor(
                        out=pv[:, w, :, :], in0=fr_v,
                        in1=lag_v[:, :, w:w + FRAME],
                        op=mybir.AluOpType.mult,
                    )
                sum_t = pool.tile([P, W * K * 40], dt.float32, tag="sum")
                sv = sum_t[:].rearrange("p (w k n) -> p w k n", w=W, k=K)
                nc.vector.tensor_tensor(
                    out=sv[:, :, :, 0:40], in0=pv[:, :, :, 0:40],
                    in1=pv[:, :, :, 40:80], op=mybir.AluOpType.add)
                nc.vector.tensor_tensor(
                    out=sv[:, :, :, 0:20], in0=sv[:, :, :, 0:20],
                    in1=sv[:, :, :, 20:40], op=mybir.AluOpType.add)
                nc.vector.tensor_tensor(
                    out=sv[:, :, :, 0:10], in0=sv[:, :, :, 0:10],
                    in1=sv[:, :, :, 10:20], op=mybir.AluOpType.add)
                # dotp in [k, w] minor order for the store
                dotp_t = pool.tile([P, K * W], dt.float32, tag="dotp")
                dotp_wv = dotp_t[:].rearrange("p (k w) -> p w k", w=W)
                nc.vector.tensor_reduce(
                    out=dotp_wv, in_=sv[:, :, :, 0:10],
                    axis=mybir.AxisListType.X, op=mybir.AluOpType.add)

                # lag energies: square then 5 sliding reduces
                lag2_t = pool.tile([P, K * WIN], dt.float32, tag="lag2")
                nc.scalar.square(lag2_t[:], lag_t[:])
                lag2_v = lag2_t[:].rearrange("p (k n) -> p k n", k=K)
                lagn_t = pool.tile([P, K * W], dt.float32, tag="lagn")
                lagn_v = lagn_t[:].rearrange("p (k w) -> p k w", w=W)
                for w in range(W):
                    nc.vector.tensor_reduce(
                        out=lagn_v[:, :, w], in_=lag2_v[:, :, w:w + FRAME],
                        axis=mybir.AxisListType.X, op=mybir.AluOpType.add)

                # frame energy
                fr2_t = pool.tile([P, K * FRAME], dt.float32, tag="fr2")
                nc.scalar.square(fr2_t[:], fr_t[:])
                frn_t = pool.tile([P, K], dt.float32, tag="frn")
                nc.vector.tensor_reduce(
                    out=frn_t[:], in_=fr2_t[:].rearrange("p (k n) -> p k n", k=K),
                    axis=mybir.AxisListType.X, op=mybir.AluOpType.add)

                # denom = frn * lagn; acorr = dotp * exp(-0.5*ln(denom+1e-9))
                den_t = pool.tile([P, K * W], dt.float32, tag="den")
                den_v = den_t[:].rearrange("p (k w) -> p k w", w=W)
                for w in range(W):
                    nc.vector.tensor_copy(out=den_v[:, :, w], in_=frn_t[:])
                nc.vector.tensor_tensor(
                    out=den_t[:], in0=den_t[:], in1=lagn_t[:],
                    op=mybir.AluOpType.mult)
                nc.scalar.activation(
                    den_t[:], den_t[:], mybir.ActivationFunctionType.Ln,
                    bias=eps_t[:])
                nc.scalar.activation(
                    den_t[:], den_t[:], mybir.ActivationFunctionType.Exp,
                    scale=-0.5)
                acorr_t = pool.tile([P, K * W], dt.float32, tag="acorr")
                nc.vector.tensor_tensor(
                    out=acorr_t[:], in0=dotp_t[:], in1=den_t[:],
                    op=mybir.AluOpType.mult)

                # store: out[g, w], g = gbase + 125*p + k
                nc.sync.dma_start(
                    out=bass.AP(tensor=out.tensor, offset=gbase * W,
                                ap=[[JSB * W, P], [1, K * W]]),
                    in_=acorr_t[:],
                )
    nc.finalize()
    return nc


def _tables():
    p = np.arange(P)[:, None]
    j = np.arange(JTOT)[None, :]
    sb, jj = j // JSB, j % JSB
    g = sb * (JSB * P) + p * JSB + jj
    r = sb * 4 + p // 32
    return (g * FRAME + 304 * r + PMAX).astype(np.int32)


def kernel(x, periods):
    from concourse.bass_utils import run_bass_kernel_spmd

    if "nc" not in _cache:
        _cache["nc"] = _build()
        _cache["cw"] = _tables()
    nc = _cache["nc"]
    cw = _cache["cw"]

    x = np.ascontiguousarray(np.asarray(x, dtype=np.float32).reshape(B, N))
    per = np.ascontiguousarray(np.asarray(periods).astype(np.int32).reshape(B, F))

    in_maps = []
    for c in range(NCORES):
        xs = x[c * BS:(c + 1) * BS]
        xp = np.zeros((BS, ROWPAD), dtype=np.float32)
        xp[:, 302:302 + N] = xs
        xp[:, 302 + N:] = xs[:, -1:]      # reference clips OOB to last sample
        in_maps.append({
            "xp": xp.reshape(-1),
            "per": per[c * BS:(c + 1) * BS].reshape(-1),
            "cw": cw,
        })

    res = run_bass_kernel_spmd(nc, in_maps, core_ids=list(range(NCORES)))
    outs = [res.results[c]["out"].reshape(BS, F, W) for c in range(NCORES)]
    return np.concatenate(outs, axis=0)[:, None, :, :]


# revision 5
# speedup vs baseline: 1.1896x; 1.1896x over previous
"""PitchAutoCorrelator Trainium2 kernel.

x [64, 1, 320000] f32, periods [64, 4000] int -> acorr [64, 1, 4000, 5] f32.

Per 80-sample frame f with period p: the 5 outputs are normalized
correlations between the frame and the lagged window
x[f*80 - p - 2 + w : ... + 80] for w in 0..4 (zero-padded at row start,
edge-clamped at row end, matching the reference's clipped gather).

Sharding: pure batch data-parallel, 8 rows per NeuronCore.
Per core: frames are blocked [128 partitions x 250 windows]; lag windows
(84 contiguous f32 at a data-dependent offset) are fetched with
one-descriptor-per-partition indirect DMAs (the only indexing form the
HW honors); dot products / energies on DVE+ACT; epilogue uses
acorr = dotp * exp(-0.5*ln(frame_nrg*lag_nrg + 1e-9)).
"""

import sys

if "/opt/trn_rl_repo" not in sys.path:
    sys.path.insert(0, "/opt/trn_rl_repo")

import numpy as np

FRAME = 80
PMAX = 300
WIN = 84          # lag window length (80 + 2*RADIUS)
W = 5             # number of lags
B, N, F = 64, 320000, 4000
NCORES = 8
BS = B // NCORES              # batch rows per core
ROWPAD = 302 + N + 2          # per-row padded length
L = BS * ROWPAD               # padded signal length per core
FPC = BS * F                  # frames per core (32000)
P = 128                       # partitions
JTOT = FPC // P               # windows per partition (250)
NSB = 2                       # super-blocks of [128 x 125]
JSB = JTOT // NSB             # 125
K = 25                        # windows per chunk
NCHUNK = JTOT // K            # 10 chunks

_cache = {}


def _build():
    import concourse.bass as bass
    import concourse.bacc as bacc
    import concourse.mybir as mybir
    import concourse.tile as tile

    dt = mybir.dt
    nc = bacc.Bacc("TRN2", target_bir_lowering=False, debug=False)
    xp = nc.dram_tensor("xp", [L], dt.float32, kind="ExternalInput")
    per = nc.dram_tensor("per", [FPC], dt.int32, kind="ExternalInput").ap()
    cw = nc.dram_tensor("cw", [P, JTOT], dt.int32, kind="ExternalInput").ap()
    out = nc.dram_tensor("out", [FPC, W], dt.float32, kind="ExternalOutput").ap()

    xp_slide = bass.AP(tensor=xp, offset=0, ap=[[1, L - WIN + 1], [1, WIN]])

    with tile.TileContext(nc) as tc:
        with tc.tile_pool(name="sbuf", bufs=2) as pool, \
             tc.tile_pool(name="cpool", bufs=1) as cpool:
            cw_t = cpool.tile([P, JTOT], dt.int32)
            nc.sync.dma_start(out=cw_t[:], in_=cw[:, :])
            eps_t = cpool.tile([P, 1], dt.float32)
            nc.vector.memset(eps_t[:], 1e-9)

            # all window offsets computed up-front so gathers never wait on
            # per-chunk DVE work
            per_t = cpool.tile([P, JTOT], dt.int32)
            for sb in range(NSB):
                nc.sync.dma_start(
                    out=per_t[:, sb * JSB:(sb + 1) * JSB],
                    in_=bass.AP(tensor=per.tensor, offset=sb * JSB * P,
                                ap=[[JSB, P], [1, JSB]]),
                )
            idx_t = cpool.tile([P, JTOT], dt.int32)
            nc.vector.tensor_tensor(
                out=idx_t[:], in0=cw_t[:], in1=per_t[:],
                op=mybir.AluOpType.subtract,
            )

            for ch in range(NCHUNK):
                sb, kk = ch // (JSB // K), ch % (JSB // K)
                jbase = sb * JSB + kk * K          # column into cw
                gbase = sb * (JSB * P) + kk * K    # frame-index base

                # lag windows: one indirect DMA per window column
                lag_t = pool.tile([P, K * WIN], dt.float32, tag="lag", bufs=3)
                for k in range(K):
                    nc.gpsimd.indirect_dma_start(
                        out=lag_t[:, k * WIN:(k + 1) * WIN],
                        out_offset=None,
                        in_=xp_slide,
                        in_offset=bass.IndirectOffsetOnAxis(
                            ap=idx_t[:, jbase + k:jbase + k + 1], axis=1),
                    )

                # frames: 4 row-group DMAs (row stride changes every 32
                # partitions)
                fr_t = pool.tile([P, K * FRAME], dt.float32, tag="fr")
                for rg in range(4):
                    g0 = sb * (JSB * P) + (32 * rg) * JSB + kk * K
                    r = sb * 4 + rg
                    off = g0 * FRAME + 304 * r + 302
                    nc.sync.dma_start(
                        out=fr_t[32 * rg:32 * rg + 32, :],
                        in_=bass.AP(tensor=xp, offset=off,
                                    ap=[[JSB * FRAME, 32], [1, K * FRAME]]),
                    )

                lag_v = lag_t[:].rearrange("p (k n) -> p k n", k=K)
                fr_v = fr_t[:].rearrange("p (k n) -> p k n", k=K)

                # dot products: 5 shifted elementwise mults + add tree
                pv_t = pool.tile([P, W * K * FRAME], dt.float32, tag="pv")
                pv = pv_t[:].rearrange("p (w k n) -> p w k n", w=W, k=K)
                for w in range(W):
                    nc.vector.tensor_tensor(
                        out=pv[:, w, :, :], in0=fr_v,
                        in1=lag_v[:, :, w:w + FRAME],
                        op=mybir.AluOpType.mult,
                    )
                sum_t = pool.tile([P, W * K * 40], dt.float32, tag="sum")
                sv = sum_t[:].rearrange("p (w k n) -> p w k n", w=W, k=K)
                nc.vector.tensor_tensor(
                    out=sv[:, :, :, 0:40], in0=pv[:, :, :, 0:40],
                    in1=pv[:, :, :, 40:80], op=mybir.AluOpType.add)
                nc.vector.tensor_tensor(
                    out=sv[:, :, :, 0:20], in0=sv[:, :, :, 0:20],
                    in1=sv[:, :, :, 20:40], op=mybir.AluOpType.add)
                nc.vector.tensor_tensor(
                    out=sv[:, :, :, 0:10], in0=sv[:, :, :, 0:10],
                    in1=sv[:, :, :, 10:20], op=mybir.AluOpType.add)
                # dotp in [k, w] minor order for the store
                dotp_t = pool.tile([P, K * W], dt.float32, tag="dotp")
                dotp_wv = dotp_t[:].rearrange("p (k w) -> p w k", w=W)
                nc.vector.tensor_reduce(
                    out=dotp_wv, in_=sv[:, :, :, 0:10],
                    axis=mybir.AxisListType.X, op=mybir.AluOpType.add)

                # lag energies: square then 5 sliding reduces
                lag2_t = pool.tile([P, K * WIN], dt.float32, tag="lag2")
                nc.scalar.square(lag2_t[:], lag_t[:])
                lag2_v = lag2_t[:].rearrange("p (k n) -> p k n", k=K)
                lagn_t = pool.tile([P, K * W], dt.float32, tag="lagn")
                lagn_v = lagn_t[:].rearrange("p (k w) -> p k w", w=W)
                nc.vector.tensor_reduce(
                    out=lagn_v[:, :, 0], in_=lag2_v[:, :, 0:FRAME],
                    axis=mybir.AxisListType.X, op=mybir.AluOpType.add)
                for w in range(1, W):
                    # lagn[w] = lagn[w-1] - lag2[w-1] + lag2[w-1+80]
                    nc.vector.tensor_tensor(
                        out=lagn_v[:, :, w], in0=lagn_v[:, :, w - 1],
                        in1=lag2_v[:, :, w - 1], op=mybir.AluOpType.subtract)
                    nc.vector.tensor_tensor(
                        out=lagn_v[:, :, w], in0=lagn_v[:, :, w],
                        in1=lag2_v[:, :, w - 1 + FRAME], op=mybir.AluOpType.add)

                # frame energy
                fr2_t = pool.tile([P, K * FRAME], dt.float32, tag="fr2")
                nc.scalar.square(fr2_t[:], fr_t[:])
                frn_t = pool.tile([P, K], dt.float32, tag="frn")
                nc.vector.tensor_reduce(
                    out=frn_t[:], in_=fr2_t[:].rearrange("p (k n) -> p k n", k=K),
                    axis=mybir.AxisListType.X, op=mybir.AluOpType.add)

                # denom = frn * lagn; acorr = dotp * exp(-0.5*ln(denom+1e-9))
                den_t = pool.tile([P, K * W], dt.float32, tag="den")
                den_v = den_t[:].rearrange("p (k w) -> p k w", w=W)
                for w in range(W):
                    nc.vector.tensor_copy(out=den_v[:, :, w], in_=frn_t[:])
                nc.vector.tensor_tensor(
                    out=den_t[:], in0=den_t[:], in1=lagn_t[:],
                    op=mybir.AluOpType.mult)
                nc.scalar.activation(
                    den_t[:], den_t[:], mybir.ActivationFunctionType.Ln,
                    bias=eps_t[:])
                nc.scalar.activation(
                    den_t[:], den_t[:], mybir.ActivationFunctionType.Exp,
                    scale=-0.5)
                acorr_t = pool.tile([P, K * W], dt.float32, tag="acorr")
                nc.vector.tensor_tensor(
                    out=acorr_t[:], in0=dotp_t[:], in1=den_t[:],
                    op=mybir.AluOpType.mult)

                # store: out[g, w], g = gbase + 125*p + k
                nc.sync.dma_start(
                    out=bass.AP(tensor=out.tensor, offset=gbase * W,
                                ap=[[JSB * W, P], [1, K * W]]),
                    in_=acorr_t[:],
                )
    nc.finalize()
    return nc


def _tables():
    p = np.arange(P)[:, None]
    j = np.arange(JTOT)[None, :]
    sb, jj = j // JSB, j % JSB
    g = sb * (JSB * P) + p * JSB + jj
    r = sb * 4 + p // 32
    return (g * FRAME + 304 * r + PMAX).astype(np.int32)


def kernel(x, periods):
    from concourse.bass_utils import run_bass_kernel_spmd

    if "nc" not in _cache:
        _cache["nc"] = _build()
        _cache["cw"] = _tables()
    nc = _cache["nc"]
    cw = _cache["cw"]

    x = np.ascontiguousarray(np.asarray(x, dtype=np.float32).reshape(B, N))
    per = np.ascontiguousarray(np.asarray(periods).astype(np.int32).reshape(B, F))

    in_maps = []
    for c in range(NCORES):
        xs = x[c * BS:(c + 1) * BS]
        xp = np.zeros((BS, ROWPAD), dtype=np.float32)
        xp[:, 302:302 + N] = xs
        xp[:, 302 + N:] = xs[:, -1:]      # reference clips OOB to last sample
        in_maps.append({
            "xp": xp.reshape(-1),
            "per": per[c * BS:(c + 1) * BS].reshape(-1),
            "cw": cw,
        })

    res = run_bass_kernel_spmd(nc, in_maps, core_ids=list(range(NCORES)))
    outs = [res.results[c]["out"].reshape(BS, F, W) for c in range(NCORES)]
    return np.concatenate(outs, axis=0)[:, None, :, :]


# revision 7
# speedup vs baseline: 1.3526x; 1.1369x over previous
"""PitchAutoCorrelator Trainium2 kernel.

x [64, 1, 320000] f32, periods [64, 4000] int -> acorr [64, 1, 4000, 5] f32.

Per 80-sample frame f with period p: the 5 outputs are normalized
correlations between the frame and the lagged window
x[f*80 - p - 2 + w : ... + 80] for w in 0..4 (zero-padded at row start,
edge-clamped at row end, matching the reference's clipped gather).

Sharding: pure batch data-parallel, 8 rows per NeuronCore.
Per core: frames are blocked [128 partitions x 250 windows]; lag windows
(84 contiguous f32 at a data-dependent offset) are fetched with
one-descriptor-per-partition indirect DMAs (the only indexing form the
HW honors); dot products / energies on DVE+ACT; epilogue uses
acorr = dotp * exp(-0.5*ln(frame_nrg*lag_nrg + 1e-9)).
"""

import sys

if "/opt/trn_rl_repo" not in sys.path:
    sys.path.insert(0, "/opt/trn_rl_repo")

import numpy as np

FRAME = 80
PMAX = 300
WIN = 84          # lag window length (80 + 2*RADIUS)
W = 5             # number of lags
B, N, F = 64, 320000, 4000
NCORES = 8
BS = B // NCORES              # batch rows per core
ROWPAD = 302 + N + 2          # per-row padded length
L = BS * ROWPAD               # padded signal length per core
FPC = BS * F                  # frames per core (32000)
P = 128                       # partitions
JTOT = FPC // P               # windows per partition (250)
NSB = 2                       # super-blocks of [128 x 125]
JSB = JTOT // NSB             # 125
K = 25                        # windows per chunk
NCHUNK = JTOT // K            # 10 chunks
USE_BF16 = True               # bf16 dot products (2x DVE); energies stay fp32

_cache = {}


def _build():
    import concourse.bass as bass
    import concourse.bacc as bacc
    import concourse.mybir as mybir
    import concourse.tile as tile

    dt = mybir.dt
    nc = bacc.Bacc("TRN2", target_bir_lowering=False, debug=False)
    xp = nc.dram_tensor("xp", [L], dt.float32, kind="ExternalInput")
    per = nc.dram_tensor("per", [FPC], dt.int32, kind="ExternalInput").ap()
    cw = nc.dram_tensor("cw", [P, JTOT], dt.int32, kind="ExternalInput").ap()
    out = nc.dram_tensor("out", [FPC, W], dt.float32, kind="ExternalOutput").ap()

    xp_slide = bass.AP(tensor=xp, offset=0, ap=[[1, L - WIN + 1], [1, WIN]])

    with tile.TileContext(nc) as tc:
        with tc.tile_pool(name="sbuf", bufs=2) as pool, \
             tc.tile_pool(name="cpool", bufs=1) as cpool:
            cw_t = cpool.tile([P, JTOT], dt.int32)
            nc.sync.dma_start(out=cw_t[:], in_=cw[:, :])
            eps_t = cpool.tile([P, 1], dt.float32)
            nc.vector.memset(eps_t[:], 1e-9)

            # all window offsets computed up-front so gathers never wait on
            # per-chunk DVE work
            per_t = cpool.tile([P, JTOT], dt.int32)
            for sb in range(NSB):
                nc.sync.dma_start(
                    out=per_t[:, sb * JSB:(sb + 1) * JSB],
                    in_=bass.AP(tensor=per.tensor, offset=sb * JSB * P,
                                ap=[[JSB, P], [1, JSB]]),
                )
            idx_t = cpool.tile([P, JTOT], dt.int32)
            nc.vector.tensor_tensor(
                out=idx_t[:], in0=cw_t[:], in1=per_t[:],
                op=mybir.AluOpType.subtract,
            )

            for ch in range(NCHUNK):
                sb, kk = ch // (JSB // K), ch % (JSB // K)
                jbase = sb * JSB + kk * K          # column into cw
                gbase = sb * (JSB * P) + kk * K    # frame-index base

                # lag windows: one indirect DMA per window column
                lag_t = pool.tile([P, K * WIN], dt.float32, tag="lag", bufs=3)
                for k in range(K):
                    nc.gpsimd.indirect_dma_start(
                        out=lag_t[:, k * WIN:(k + 1) * WIN],
                        out_offset=None,
                        in_=xp_slide,
                        in_offset=bass.IndirectOffsetOnAxis(
                            ap=idx_t[:, jbase + k:jbase + k + 1], axis=1),
                    )

                # frames: 4 row-group DMAs (row stride changes every 32
                # partitions)
                fr_t = pool.tile([P, K * FRAME], dt.float32, tag="fr")
                for rg in range(4):
                    g0 = sb * (JSB * P) + (32 * rg) * JSB + kk * K
                    r = sb * 4 + rg
                    off = g0 * FRAME + 304 * r + 302
                    nc.sync.dma_start(
                        out=fr_t[32 * rg:32 * rg + 32, :],
                        in_=bass.AP(tensor=xp, offset=off,
                                    ap=[[JSB * FRAME, 32], [1, K * FRAME]]),
                    )

                lag_v = lag_t[:].rearrange("p (k n) -> p k n", k=K)
                fr_v = fr_t[:].rearrange("p (k n) -> p k n", k=K)

                if USE_BF16:
                    # bf16 copies: frames, lag (even shifts), lag+1 (odd
                    # shifts keep 4B alignment for the DVE 2x mode)
                    frb_t = pool.tile([P, K * FRAME], dt.bfloat16, tag="frb")
                    nc.vector.tensor_copy(out=frb_t[:], in_=fr_t[:])
                    lagb_t = pool.tile([P, K * WIN], dt.bfloat16, tag="lagb")
                    nc.vector.tensor_copy(out=lagb_t[:], in_=lag_t[:])
                    lags_t = pool.tile([P, K * WIN], dt.bfloat16, tag="lags")
                    nc.vector.tensor_copy(
                        out=lags_t[:, 0:K * WIN - 1], in_=lagb_t[:, 1:])
                    mul_in0 = frb_t[:].rearrange("p (k n) -> p k n", k=K)
                    lv_e = lagb_t[:].rearrange("p (k n) -> p k n", k=K)
                    lv_o = lags_t[:].rearrange("p (k n) -> p k n", k=K)
                    pdt = dt.bfloat16
                else:
                    mul_in0 = fr_v
                    lv_e = lag_v
                    pdt = dt.float32

                # dot products: 5 shifted elementwise mults + add tree
                pv_t = pool.tile([P, W * K * FRAME], pdt, tag="pv")
                pv = pv_t[:].rearrange("p (w k n) -> p w k n", w=W, k=K)
                for w in range(W):
                    if USE_BF16:
                        src = lv_e if w % 2 == 0 else lv_o
                        off = w // 2 * 2 if w % 2 == 0 else (w - 1)
                        in1 = src[:, :, off:off + FRAME]
                    else:
                        in1 = lv_e[:, :, w:w + FRAME]
                    nc.vector.tensor_tensor(
                        out=pv[:, w, :, :], in0=mul_in0, in1=in1,
                        op=mybir.AluOpType.mult,
                    )
                sum_t = pool.tile([P, W * K * 40], pdt, tag="sum")
                sv = sum_t[:].rearrange("p (w k n) -> p w k n", w=W, k=K)
                nc.vector.tensor_tensor(
                    out=sv[:, :, :, 0:40], in0=pv[:, :, :, 0:40],
                    in1=pv[:, :, :, 40:80], op=mybir.AluOpType.add)
                nc.vector.tensor_tensor(
                    out=sv[:, :, :, 0:20], in0=sv[:, :, :, 0:20],
                    in1=sv[:, :, :, 20:40], op=mybir.AluOpType.add)
                nc.vector.tensor_tensor(
                    out=sv[:, :, :, 0:10], in0=sv[:, :, :, 0:10],
                    in1=sv[:, :, :, 10:20], op=mybir.AluOpType.add)
                # dotp in [k, w] minor order for the store
                dotp_t = pool.tile([P, K * W], dt.float32, tag="dotp")
                dotp_wv = dotp_t[:].rearrange("p (k w) -> p w k", w=W)
                nc.vector.tensor_reduce(
                    out=dotp_wv, in_=sv[:, :, :, 0:10],
                    axis=mybir.AxisListType.X, op=mybir.AluOpType.add)

                # lag energies: square then 5 sliding reduces
                lag2_t = pool.tile([P, K * WIN], dt.float32, tag="lag2")
                nc.scalar.square(lag2_t[:], lag_t[:])
                lag2_v = lag2_t[:].rearrange("p (k n) -> p k n", k=K)
                lagn_t = pool.tile([P, K * W], dt.float32, tag="lagn")
                lagn_v = lagn_t[:].rearrange("p (k w) -> p k w", w=W)
                nc.vector.tensor_reduce(
                    out=lagn_v[:, :, 0], in_=lag2_v[:, :, 0:FRAME],
                    axis=mybir.AxisListType.X, op=mybir.AluOpType.add)
                for w in range(1, W):
                    # lagn[w] = lagn[w-1] - lag2[w-1] + lag2[w-1+80]
                    nc.vector.tensor_tensor(
                        out=lagn_v[:, :, w], in0=lagn_v[:, :, w - 1],
                        in1=lag2_v[:, :, w - 1], op=mybir.AluOpType.subtract)
                    nc.vector.tensor_tensor(
                        out=lagn_v[:, :, w], in0=lagn_v[:, :, w],
                        in1=lag2_v[:, :, w - 1 + FRAME], op=mybir.AluOpType.add)

                # frame energy
                fr2_t = pool.tile([P, K * FRAME], dt.float32, tag="fr2")
                nc.scalar.square(fr2_t[:], fr_t[:])
                frn_t = pool.tile([P, K], dt.float32, tag="frn")
                nc.vector.tensor_reduce(
                    out=frn_t[:], in_=fr2_t[:].rearrange("p (k n) -> p k n", k=K),
                    axis=mybir.AxisListType.X, op=mybir.AluOpType.add)

                # denom = frn * lagn; acorr = dotp * exp(-0.5*ln(denom+1e-9))
                den_t = pool.tile([P, K * W], dt.float32, tag="den")
                den_v = den_t[:].rearrange("p (k w) -> p k w", w=W)
                for w in range(W):
                    nc.vector.tensor_copy(out=den_v[:, :, w], in_=frn_t[:])
                nc.vector.tensor_tensor(
                    out=den_t[:], in0=den_t[:], in1=lagn_t[:],
                    op=mybir.AluOpType.mult)
                nc.scalar.activation(
                    den_t[:], den_t[:], mybir.ActivationFunctionType.Ln,
                    bias=eps_t[:])
                nc.scalar.activation(
                    den_t[:], den_t[:], mybir.ActivationFunctionType.Exp,
                    scale=-0.5)
                acorr_t = pool.tile([P, K * W], dt.float32, tag="acorr")
                nc.vector.tensor_tensor(
                    out=acorr_t[:], in0=dotp_t[:], in1=den_t[:],
                    op=mybir.AluOpType.mult)

                # store: out[g, w], g = gbase + 125*p + k
                nc.sync.dma_start(
                    out=bass.AP(tensor=out.tensor, offset=gbase * W,
                                ap=[[JSB * W, P], [1, K * W]]),
                    in_=acorr_t[:],
                )
    nc.finalize()
    return nc


def _tables():
    p = np.arange(P)[:, None]
    j = np.arange(JTOT)[None, :]
    sb, jj = j // JSB, j % JSB
    g = sb * (JSB * P) + p * JSB + jj
    r = sb * 4 + p // 32
    return (g * FRAME + 304 * r + PMAX).astype(np.int32)


def kernel(x, periods):
    from concourse.bass_utils import run_bass_kernel_spmd

    if "nc" not in _cache:
        _cache["nc"] = _build()
        _cache["cw"] = _tables()
    nc = _cache["nc"]
    cw = _cache["cw"]

    x = np.ascontiguousarray(np.asarray(x, dtype=np.float32).reshape(B, N))
    per = np.ascontiguousarray(np.asarray(periods).astype(np.int32).reshape(B, F))

    in_maps = []
    for c in range(NCORES):
        xs = x[c * BS:(c + 1) * BS]
        xp = np.zeros((BS, ROWPAD), dtype=np.float32)
        xp[:, 302:302 + N] = xs
        xp[:, 302 + N:] = xs[:, -1:]      # reference clips OOB to last sample
        in_maps.append({
            "xp": xp.reshape(-1),
            "per": per[c * BS:(c + 1) * BS].reshape(-1),
            "cw": cw,
        })

    res = run_bass_kernel_spmd(nc, in_maps, core_ids=list(range(NCORES)))
    outs = [res.results[c]["out"].reshape(BS, F, W) for c in range(NCORES)]
    return np.concatenate(outs, axis=0)[:, None, :, :]
